# revision 5
# baseline (speedup 1.0000x reference)
"""Self-contained Trainium2 Bass kernel for a 3-layer MPNN (N=50000, E=800000, D=64).

Math: each layer is
    x' = relu(concat(segment_sum(x[src]@Wm+bm, dst), x) @ Wu + bu)
with self-loops added. Since the message fn is linear this folds to
    T    = x @ (Wm @ Wu[:D])                      (per-node table)
    y[v] = sum_{e: dst=v} T[src_e]                (scatter-add, no self-loop)
    x'   = relu(y + x @ (Wm@Wu[:D] + Wu[D:]) + deg*(bm@Wu[:D]) + bu)

Sharding: nodes padded to 50176 = 8*6272; core k owns nodes [6272k, 6272(k+1)),
as 49 blocks of 128. Host sorts each core's incident edges (by dst) into
128-edge chunks per dst-block. Device: indirect-DMA gather of T rows, one-hot
matmul scatter-add accumulated in PSUM, dense epilogue matmuls, AllGather of
the next layer's T table.
"""
import numpy as np
from contextlib import ExitStack

import concourse.bass as bass
import concourse.bacc as bacc
import concourse.mybir as mybir
import concourse.tile as tile
from concourse.bass_utils import run_bass_kernel_spmd
from concourse.masks import make_identity

N = 50000
E = 800000
D = 64
NCORE = 8
P = 128
PERCORE = 6272          # 49 * 128
NPAD = PERCORE * NCORE  # 50176
NBLK = PERCORE // P     # 49
F32 = mybir.dt.float32
I32 = mybir.dt.int32


def _preprocess(edge_index):
    """Partition + sort edges; returns per-core packed chunk arrays and counts.

    src_packT / dst_packT: [NCORE, P, TOT] — column base+c holds chunk c of the
    block whose chunk range is [starts[b], starts[b]+cnts[b]); row p is edge
    slot p of that chunk.
    """
    src = edge_index[0].astype(np.int64)
    dst = edge_index[1].astype(np.int64)
    core = dst // PERCORE
    blk = (dst % PERCORE) // P
    loc = dst % P

    deg = np.bincount(dst, minlength=NPAD).astype(np.float32) + 1.0

    cnt_kb = np.zeros((NCORE, NBLK), dtype=np.int64)
    order = np.lexsort((loc, blk, core))
    src_s, core_s, blk_s, loc_s = src[order], core[order], blk[order], loc[order]
    np.add.at(cnt_kb, (core_s, blk_s), 1)
    chunks_kb = (cnt_kb + P - 1) // P
    cnts = np.maximum(chunks_kb.max(axis=0), 1)  # per-block chunk count (shared)
    starts = np.concatenate([[0], np.cumsum(cnts)])
    TOT = int(starts[-1])

    src_packT = np.zeros((NCORE, P, TOT), dtype=np.int32)
    dst_packT = np.full((NCORE, P, TOT), -1.0, dtype=np.float32)

    flat_cnt = cnt_kb.ravel()
    flat_start = np.concatenate([[0], np.cumsum(flat_cnt)])[:-1].reshape(NCORE, NBLK)
    for k in range(NCORE):
        for b in range(NBLK):
            n = int(cnt_kb[k, b])
            if n == 0:
                continue
            s0 = int(flat_start[k, b])
            base = int(starts[b])
            w = int(cnts[b])
            buf_s = np.zeros((w * P,), dtype=np.int32)
            buf_d = np.full((w * P,), -1.0, dtype=np.float32)
            buf_s[:n] = src_s[s0:s0 + n]
            buf_d[:n] = loc_s[s0:s0 + n]
            # chunk-major -> [P, w] (edge slot p of chunk c at [p, c])
            src_packT[k, :, base:base + w] = buf_s.reshape(w, P).T
            dst_packT[k, :, base:base + w] = buf_d.reshape(w, P).T
    return src_packT, dst_packT, deg, cnts.astype(int), starts.astype(int), TOT


def _build(cnts, starts, TOT):
    nc = bacc.Bacc("TRN2", target_bir_lowering=False, debug=False,
                   num_devices=NCORE)
    x_own = nc.dram_tensor("x_own", [PERCORE, D], F32, kind="ExternalInput")
    src_idx = nc.dram_tensor("src_idx", [P, TOT], I32, kind="ExternalInput")
    dst_loc = nc.dram_tensor("dst_loc", [P, TOT], F32, kind="ExternalInput")
    deg2 = nc.dram_tensor("deg2", [2, PERCORE], F32, kind="ExternalInput")
    iota_in = nc.dram_tensor("iota_in", [P, P], F32, kind="ExternalInput")
    Wm_in = nc.dram_tensor("Wm_in", [3, D, D], F32, kind="ExternalInput")
    Wu_in = nc.dram_tensor("Wu_in", [3, 2 * D, D], F32, kind="ExternalInput")
    bm_in = nc.dram_tensor("bm_in", [3, D], F32, kind="ExternalInput")
    bu_in = nc.dram_tensor("bu_in", [3, D], F32, kind="ExternalInput")
    out = nc.dram_tensor("out", [PERCORE, D], F32, kind="ExternalOutput")

    T_own = [nc.dram_tensor(f"T_own{l}", [PERCORE, D], F32) for l in range(3)]
    T_full = [nc.dram_tensor(f"T_full{l}", [NPAD, D], F32, addr_space="Shared")
              for l in range(3)]
    groups = [list(range(NCORE))]

    with tile.TileContext(nc) as tc, ExitStack() as ctx:
        const = ctx.enter_context(tc.tile_pool(name="const", bufs=1))
        sb = ctx.enter_context(tc.tile_pool(name="sb", bufs=4))
        gat = ctx.enter_context(tc.tile_pool(name="gat", bufs=8))
        oneh = ctx.enter_context(tc.tile_pool(name="oneh", bufs=4))
        ps_y = ctx.enter_context(tc.tile_pool(name="ps_y", bufs=2, space="PSUM"))
        ps_m = ctx.enter_context(tc.tile_pool(name="ps_m", bufs=3, space="PSUM"))
        ps_w = ctx.enter_context(tc.tile_pool(name="ps_w", bufs=1, space="PSUM"))

        ident = const.tile([P, P], F32)
        make_identity(nc, ident[:])
        iota = const.tile([P, P], F32)
        nc.sync.dma_start(out=iota[:], in_=iota_in[:])

        # index/dst-local tables resident in SBUF (one big DMA each)
        src_all = const.tile([P, TOT], I32, tag="src_all")
        nc.sync.dma_start(out=src_all[:], in_=src_idx[:])
        dst_all = const.tile([P, TOT], F32, tag="dst_all")
        nc.sync.dma_start(out=dst_all[:], in_=dst_loc[:])
        degt = const.tile([2, PERCORE], F32, tag="degt")
        nc.sync.dma_start(out=degt[:], in_=deg2[:])

        # --- per-layer weight prep: W1 = Wm@Wu_top, W2 = W1 + Wu_bot,
        # bias_rhs = [bm@Wu_top ; bu] ---
        W1s, W2s, biasr = [], [], []
        for l in range(3):
            wm = const.tile([D, D], F32, tag=f"wm{l}")
            nc.sync.dma_start(out=wm[:], in_=Wm_in[l])
            wu_t = const.tile([D, D], F32, tag=f"wut{l}")
            nc.sync.dma_start(out=wu_t[:], in_=Wu_in[l, :D])
            wu_b = const.tile([D, D], F32, tag=f"wub{l}")
            nc.sync.dma_start(out=wu_b[:], in_=Wu_in[l, D:])
            wmT_ps = ps_w.tile([D, D], F32, tag="psw")
            nc.tensor.transpose(out=wmT_ps[:], in_=wm[:], identity=ident[:D, :D])
            wmT = const.tile([D, D], F32, tag=f"wmT{l}")
            nc.vector.tensor_copy(out=wmT[:], in_=wmT_ps[:])
            w1_ps = ps_w.tile([D, D], F32, tag="psw")
            nc.tensor.matmul(out=w1_ps[:], lhsT=wmT[:], rhs=wu_t[:],
                             start=True, stop=True)
            w1 = const.tile([D, D], F32, tag=f"w1{l}")
            nc.vector.tensor_copy(out=w1[:], in_=w1_ps[:])
            w2 = const.tile([D, D], F32, tag=f"w2{l}")
            nc.vector.tensor_add(out=w2[:], in0=w1_ps[:], in1=wu_b[:])
            bmc = const.tile([D, 1], F32, tag=f"bmc{l}")
            nc.sync.dma_start(out=bmc[:], in_=bm_in[l][:, None])
            b1_ps = ps_w.tile([1, D], F32, tag="psw")
            nc.tensor.matmul(out=b1_ps[:], lhsT=bmc[:], rhs=wu_t[:],
                             start=True, stop=True)
            br = const.tile([2, D], F32, tag=f"br{l}")
            nc.vector.tensor_copy(out=br[:1, :], in_=b1_ps[:])
            nc.sync.dma_start(out=br[1:2, :], in_=bu_in[l][None, :])
            W1s.append(w1)
            W2s.append(w2)
            biasr.append(br)

        # persistent transposed-x buffers (ping-pong across layers)
        xT = [const.tile([D, PERCORE], F32, tag=f"xT{i}", name=f"xT{i}")
              for i in range(2)]

        # --- layer 0 table: T0 = x_own @ W1_0 (+ build xT[0]) ---
        for b in range(NBLK):
            bs, be = b * P, (b + 1) * P
            xb = sb.tile([P, D], F32, tag="xb0")
            nc.sync.dma_start(out=xb[:], in_=x_own[bs:be, :])
            xT_ps = ps_m.tile([D, P], F32, tag="psm")
            nc.tensor.transpose(out=xT_ps[:], in_=xb[:], identity=ident[:])
            nc.vector.tensor_copy(out=xT[0][:, bs:be], in_=xT_ps[:])
            t0_ps = ps_m.tile([P, D], F32, tag="psm")
            nc.tensor.matmul(out=t0_ps[:], lhsT=xT[0][:, bs:be], rhs=W1s[0][:],
                             start=True, stop=True)
            t0 = sb.tile([P, D], F32, tag="t0sb")
            nc.vector.tensor_copy(out=t0[:], in_=t0_ps[:])
            nc.sync.dma_start(out=T_own[0][bs:be, :], in_=t0[:])
        nc.gpsimd.collective_compute(
            "AllGather", mybir.AluOpType.bypass, replica_groups=groups,
            ins=[T_own[0][:]], outs=[T_full[0][:]])

        # --- 3 layers ---
        for l in range(3):
            xT_cur = xT[l % 2]
            xT_nxt = xT[(l + 1) % 2]
            for b in range(NBLK):
                bs, be = b * P, (b + 1) * P
                nch = int(cnts[b])
                base = int(starts[b])
                psy = ps_y.tile([P, D], F32, tag="psy")
                for c in range(nch):
                    g = gat.tile([P, D], F32, tag="g")
                    nc.gpsimd.indirect_dma_start(
                        out=g[:], out_offset=None,
                        in_=T_full[l][:],
                        in_offset=bass.IndirectOffsetOnAxis(
                            ap=src_all[:, base + c:base + c + 1], axis=0))
                    oh = oneh.tile([P, P], F32, tag="oh")
                    nc.vector.tensor_tensor(
                        out=oh[:],
                        in0=dst_all[:, base + c:base + c + 1].to_broadcast([P, P])[:],
                        in1=iota[:], op=mybir.AluOpType.is_equal)
                    nc.tensor.matmul(out=psy[:], lhsT=oh[:], rhs=g[:],
                                     start=(c == 0), stop=False)
                nc.tensor.matmul(out=psy[:], lhsT=xT_cur[:, bs:be],
                                 rhs=W2s[l][:], start=False, stop=False)
                nc.tensor.matmul(out=psy[:], lhsT=degt[:, bs:be], rhs=biasr[l][:],
                                 start=False, stop=True)
                xnew = sb.tile([P, D], F32, tag="xnew")
                nc.scalar.activation(out=xnew[:], in_=psy[:],
                                     func=mybir.ActivationFunctionType.Relu)
                if l == 2:
                    nc.sync.dma_start(out=out[bs:be, :], in_=xnew[:])
                else:
                    xT_ps = ps_m.tile([D, P], F32, tag="psm")
                    nc.tensor.transpose(out=xT_ps[:], in_=xnew[:],
                                        identity=ident[:])
                    nc.vector.tensor_copy(out=xT_nxt[:, bs:be], in_=xT_ps[:])
                    tn_ps = ps_m.tile([P, D], F32, tag="psm")
                    nc.tensor.matmul(out=tn_ps[:], lhsT=xT_nxt[:, bs:be],
                                     rhs=W1s[l + 1][:], start=True, stop=True)
                    tn = sb.tile([P, D], F32, tag="tnsb")
                    nc.vector.tensor_copy(out=tn[:], in_=tn_ps[:])
                    nc.sync.dma_start(out=T_own[l + 1][bs:be, :], in_=tn[:])
            if l < 2:
                nc.gpsimd.collective_compute(
                    "AllGather", mybir.AluOpType.bypass, replica_groups=groups,
                    ins=[T_own[l + 1][:]], outs=[T_full[l + 1][:]])

    nc.compile()
    return nc


def prepare(x, edge_index, Wm0, bm0, Wu0, bu0, Wm1, bm1, Wu1, bu1,
            Wm2, bm2, Wu2, bu2):
    """Returns (nc, in_maps) ready for run_bass_kernel_spmd."""
    x = np.asarray(x, dtype=np.float32)
    edge_index = np.asarray(edge_index)
    src_packT, dst_packT, deg, cnts, starts, TOT = _preprocess(edge_index)

    xpad = np.zeros((NPAD, D), dtype=np.float32)
    xpad[:N] = x
    iota = np.broadcast_to(np.arange(P, dtype=np.float32), (P, P)).copy()
    Wm = np.stack([np.asarray(w, np.float32) for w in (Wm0, Wm1, Wm2)])
    Wu = np.stack([np.asarray(w, np.float32) for w in (Wu0, Wu1, Wu2)])
    bm = np.stack([np.asarray(w, np.float32) for w in (bm0, bm1, bm2)])
    bu = np.stack([np.asarray(w, np.float32) for w in (bu0, bu1, bu2)])

    in_maps = []
    for k in range(NCORE):
        deg_k = deg[k * PERCORE:(k + 1) * PERCORE]
        deg2 = np.stack([deg_k, np.ones_like(deg_k)], axis=0)  # [2, PERCORE]
        in_maps.append({
            "x_own": np.ascontiguousarray(xpad[k * PERCORE:(k + 1) * PERCORE]),
            "src_idx": src_packT[k],
            "dst_loc": dst_packT[k],
            "deg2": np.ascontiguousarray(deg2),
            "iota_in": iota,
            "Wm_in": Wm, "Wu_in": Wu, "bm_in": bm, "bu_in": bu,
        })

    nc = _build(cnts, starts, TOT)
    return nc, in_maps


def kernel(**inputs):
    nc, in_maps = prepare(**inputs)
    res = run_bass_kernel_spmd(nc, in_maps, list(range(NCORE)))
    full = np.concatenate([res.results[k]["out"] for k in range(NCORE)], axis=0)
    return np.ascontiguousarray(full[:N])


# revision 8
# speedup vs baseline: 1.2321x; 1.2321x over previous
"""Self-contained Trainium2 Bass kernel for a 3-layer MPNN (N=50000, E=800000, D=64).

Math: each layer is
    x' = relu(concat(segment_sum(x[src]@Wm+bm, dst), x) @ Wu + bu)
with self-loops added. Since the message fn is linear this folds to
    T    = x @ (Wm @ Wu[:D])                      (per-node table)
    y[v] = sum_{e: dst=v} T[src_e]                (scatter-add, no self-loop)
    x'   = relu(y + x @ (Wm@Wu[:D] + Wu[D:]) + deg*(bm@Wu[:D]) + bu)

Sharding: nodes padded to 50176 = 8*6272; core k owns nodes [6272k, 6272(k+1)),
as 49 blocks of 128. Host sorts each core's incident edges by (dst block,
src-half) into 128-edge chunks. Device: batched dma_gather of T rows (one op
per block-half; int16 indices force the src-half table split), one-hot matmul
scatter-add accumulated in PSUM, dense epilogue matmuls, AllGather of the next
layer's T table.
"""
import numpy as np
from contextlib import ExitStack

import concourse.bass as bass
import concourse.bacc as bacc
import concourse.mybir as mybir
import concourse.tile as tile
from concourse.bass_utils import run_bass_kernel_spmd
from concourse.masks import make_identity

N = 50000
E = 800000
D = 64
NCORE = 8
P = 128
PERCORE = 6272          # 49 * 128
NPAD = PERCORE * NCORE  # 50176
NBLK = PERCORE // P     # 49
HALF = 32768            # int16 index limit for dma_gather
F32 = mybir.dt.float32
I16 = mybir.dt.int16


def _preprocess(edge_index):
    """Partition + sort edges by (dst-core, dst-block, src-half).

    Returns:
      gidx  [NCORE, P, 8*TOT]  int16 wrapped gather indices (idx i of a run at
                               [i%16 (replicated mod 16), run_col0 + i//16])
      dstl  [NCORE, P, TOT]    f32 dst-local (edge slot p of chunk col c), -1 pad
      deg   [NPAD]             f32 in-degree + 1
      cnts2 [NBLK, 2]          chunks per (block, half), shared across cores
      starts2 [NBLK, 2]        chunk-column starts
      TOT   total chunk columns
    """
    src = edge_index[0].astype(np.int64)
    dst = edge_index[1].astype(np.int64)
    core = dst // PERCORE
    blk = (dst % PERCORE) // P
    loc = dst % P
    half = (src >= HALF).astype(np.int64)

    deg = np.bincount(dst, minlength=NPAD).astype(np.float32) + 1.0

    order = np.lexsort((loc, half, blk, core))
    src_s, loc_s = src[order], loc[order]
    core_s, blk_s, half_s = core[order], blk[order], half[order]

    cnt = np.zeros((NCORE, NBLK, 2), dtype=np.int64)
    np.add.at(cnt, (core_s, blk_s, half_s), 1)
    cnts2 = (cnt + P - 1) // P
    cnts2 = cnts2.max(axis=0)  # [NBLK, 2]
    flat = cnts2.reshape(-1)
    starts_flat = np.concatenate([[0], np.cumsum(flat)])
    TOT = int(starts_flat[-1])
    starts2 = starts_flat[:-1].reshape(NBLK, 2)

    gidx = np.zeros((NCORE, P, 8 * TOT), dtype=np.int16)
    dstl = np.full((NCORE, P, TOT), -1.0, dtype=np.float32)

    run_start = np.concatenate([[0], np.cumsum(cnt.ravel())])[:-1].reshape(
        NCORE, NBLK, 2)
    for k in range(NCORE):
        for b in range(NBLK):
            for h in range(2):
                n = int(cnt[k, b, h])
                w = int(cnts2[b, h])
                if w == 0:
                    continue
                st = int(starts2[b, h])
                ridx = np.zeros((w * P,), dtype=np.int64)
                rloc = np.full((w * P,), -1.0, dtype=np.float32)
                if n:
                    s0 = int(run_start[k, b, h])
                    ridx[:n] = src_s[s0:s0 + n] - h * HALF
                    rloc[:n] = loc_s[s0:s0 + n]
                # wrapped idx: [16, w*8] -> replicate to 128 partitions
                w16 = ridx.reshape(w * 8, 16).T.astype(np.int16)
                gidx[k, :, 8 * st:8 * (st + w)] = np.tile(w16, (8, 1))
                dstl[k, :, st:st + w] = rloc.reshape(w, P).T
    return gidx, dstl, deg, cnts2.astype(int), starts2.astype(int), TOT


def _build(cnts2, starts2, TOT):
    nc = bacc.Bacc("TRN2", target_bir_lowering=False, debug=False,
                   num_devices=NCORE)
    x_own = nc.dram_tensor("x_own", [PERCORE, D], F32, kind="ExternalInput")
    gidx_in = nc.dram_tensor("gidx", [P, 8 * TOT], I16, kind="ExternalInput")
    dst_loc = nc.dram_tensor("dst_loc", [P, TOT], F32, kind="ExternalInput")
    deg2 = nc.dram_tensor("deg2", [2, PERCORE], F32, kind="ExternalInput")
    iota_in = nc.dram_tensor("iota_in", [P, P], F32, kind="ExternalInput")
    Wm_in = nc.dram_tensor("Wm_in", [3, D, D], F32, kind="ExternalInput")
    Wu_in = nc.dram_tensor("Wu_in", [3, 2 * D, D], F32, kind="ExternalInput")
    bm_in = nc.dram_tensor("bm_in", [3, D], F32, kind="ExternalInput")
    bu_in = nc.dram_tensor("bu_in", [3, D], F32, kind="ExternalInput")
    out = nc.dram_tensor("out", [PERCORE, D], F32, kind="ExternalOutput")

    T_own = [nc.dram_tensor(f"T_own{l}", [PERCORE, D], F32) for l in range(3)]
    T_full = [nc.dram_tensor(f"T_full{l}", [NPAD, D], F32, addr_space="Shared")
              for l in range(3)]
    groups = [list(range(NCORE))]

    with tile.TileContext(nc) as tc, ExitStack() as ctx:
        const = ctx.enter_context(tc.tile_pool(name="const", bufs=1))
        sb = ctx.enter_context(tc.tile_pool(name="sb", bufs=4))
        gat = ctx.enter_context(tc.tile_pool(name="gat", bufs=4))
        oneh = ctx.enter_context(tc.tile_pool(name="oneh", bufs=6))
        ps_y = ctx.enter_context(tc.tile_pool(name="ps_y", bufs=2, space="PSUM"))
        ps_m = ctx.enter_context(tc.tile_pool(name="ps_m", bufs=3, space="PSUM"))
        ps_w = ctx.enter_context(tc.tile_pool(name="ps_w", bufs=1, space="PSUM"))

        ident = const.tile([P, P], F32)
        make_identity(nc, ident[:])
        iota = const.tile([P, P], F32)
        nc.sync.dma_start(out=iota[:], in_=iota_in[:])

        gidx_sb = const.tile([P, 8 * TOT], I16, tag="gidx_sb")
        nc.sync.dma_start(out=gidx_sb[:], in_=gidx_in[:])
        dst_all = const.tile([P, TOT], F32, tag="dst_all")
        nc.sync.dma_start(out=dst_all[:], in_=dst_loc[:])
        degt = const.tile([2, PERCORE], F32, tag="degt")
        nc.sync.dma_start(out=degt[:], in_=deg2[:])

        # --- per-layer weight prep: W1 = Wm@Wu_top, W2 = W1 + Wu_bot,
        # bias_rhs = [bm@Wu_top ; bu] ---
        W1s, W2s, biasr = [], [], []
        for l in range(3):
            wm = const.tile([D, D], F32, tag=f"wm{l}")
            nc.sync.dma_start(out=wm[:], in_=Wm_in[l])
            wu_t = const.tile([D, D], F32, tag=f"wut{l}")
            nc.sync.dma_start(out=wu_t[:], in_=Wu_in[l, :D])
            wu_b = const.tile([D, D], F32, tag=f"wub{l}")
            nc.sync.dma_start(out=wu_b[:], in_=Wu_in[l, D:])
            wmT_ps = ps_w.tile([D, D], F32, tag="psw")
            nc.tensor.transpose(out=wmT_ps[:], in_=wm[:], identity=ident[:D, :D])
            wmT = const.tile([D, D], F32, tag=f"wmT{l}")
            nc.vector.tensor_copy(out=wmT[:], in_=wmT_ps[:])
            w1_ps = ps_w.tile([D, D], F32, tag="psw")
            nc.tensor.matmul(out=w1_ps[:], lhsT=wmT[:], rhs=wu_t[:],
                             start=True, stop=True)
            w1 = const.tile([D, D], F32, tag=f"w1{l}")
            nc.vector.tensor_copy(out=w1[:], in_=w1_ps[:])
            w2 = const.tile([D, D], F32, tag=f"w2{l}")
            nc.vector.tensor_add(out=w2[:], in0=w1_ps[:], in1=wu_b[:])
            bmc = const.tile([D, 1], F32, tag=f"bmc{l}")
            nc.sync.dma_start(out=bmc[:], in_=bm_in[l][:, None])
            b1_ps = ps_w.tile([1, D], F32, tag="psw")
            nc.tensor.matmul(out=b1_ps[:], lhsT=bmc[:], rhs=wu_t[:],
                             start=True, stop=True)
            br = const.tile([2, D], F32, tag=f"br{l}")
            nc.vector.tensor_copy(out=br[:1, :], in_=b1_ps[:])
            nc.sync.dma_start(out=br[1:2, :], in_=bu_in[l][None, :])
            W1s.append(w1)
            W2s.append(w2)
            biasr.append(br)

        # persistent transposed-x buffers (ping-pong across layers)
        xT = [const.tile([D, PERCORE], F32, tag=f"xT{i}", name=f"xT{i}")
              for i in range(2)]

        # --- layer 0 table: T0 = x_own @ W1_0 (+ build xT[0]) ---
        for b in range(NBLK):
            bs, be = b * P, (b + 1) * P
            xb = sb.tile([P, D], F32, tag="xb0")
            nc.sync.dma_start(out=xb[:], in_=x_own[bs:be, :])
            xT_ps = ps_m.tile([D, P], F32, tag="psm")
            nc.tensor.transpose(out=xT_ps[:], in_=xb[:], identity=ident[:])
            nc.vector.tensor_copy(out=xT[0][:, bs:be], in_=xT_ps[:])
            t0_ps = ps_m.tile([P, D], F32, tag="psm")
            nc.tensor.matmul(out=t0_ps[:], lhsT=xT[0][:, bs:be], rhs=W1s[0][:],
                             start=True, stop=True)
            t0 = sb.tile([P, D], F32, tag="t0sb")
            nc.vector.tensor_copy(out=t0[:], in_=t0_ps[:])
            nc.sync.dma_start(out=T_own[0][bs:be, :], in_=t0[:])
        nc.gpsimd.collective_compute(
            "AllGather", mybir.AluOpType.bypass, replica_groups=groups,
            ins=[T_own[0][:]], outs=[T_full[0][:]])

        # --- 3 layers ---
        for l in range(3):
            xT_cur = xT[l % 2]
            xT_nxt = xT[(l + 1) % 2]
            for b in range(NBLK):
                bs, be = b * P, (b + 1) * P
                psy = ps_y.tile([P, D], F32, tag="psy")
                first = True
                for h in range(2):
                    c2 = int(cnts2[b, h])
                    if c2 == 0:
                        continue
                    st = int(starts2[b, h])
                    rows = HALF if h == 0 else NPAD - HALF
                    gt = gat.tile([P, c2, D], F32, tag="g")
                    nc.gpsimd.dma_gather(
                        out_ap=gt[:],
                        in_ap=T_full[l][h * HALF:h * HALF + rows, :],
                        idxs_ap=gidx_sb[:, 8 * st:8 * (st + c2)],
                        num_idxs=c2 * P,
                        num_idxs_reg=c2 * P,
                        elem_size=D,
                        single_packet=False,
                    )
                    for c in range(c2):
                        oh = oneh.tile([P, P], F32, tag="oh")
                        nc.vector.tensor_tensor(
                            out=oh[:],
                            in0=dst_all[:, st + c:st + c + 1]
                                .to_broadcast([P, P])[:],
                            in1=iota[:], op=mybir.AluOpType.is_equal)
                        nc.tensor.matmul(out=psy[:], lhsT=oh[:],
                                         rhs=gt[:, c, :],
                                         start=first, stop=False)
                        first = False
                nc.tensor.matmul(out=psy[:], lhsT=xT_cur[:, bs:be],
                                 rhs=W2s[l][:], start=first, stop=False)
                nc.tensor.matmul(out=psy[:], lhsT=degt[:, bs:be],
                                 rhs=biasr[l][:], start=False, stop=True)
                xnew = sb.tile([P, D], F32, tag="xnew")
                nc.scalar.activation(out=xnew[:], in_=psy[:],
                                     func=mybir.ActivationFunctionType.Relu)
                if l == 2:
                    nc.sync.dma_start(out=out[bs:be, :], in_=xnew[:])
                else:
                    xT_ps = ps_m.tile([D, P], F32, tag="psm")
                    nc.tensor.transpose(out=xT_ps[:], in_=xnew[:],
                                        identity=ident[:])
                    nc.vector.tensor_copy(out=xT_nxt[:, bs:be], in_=xT_ps[:])
                    tn_ps = ps_m.tile([P, D], F32, tag="psm")
                    nc.tensor.matmul(out=tn_ps[:], lhsT=xT_nxt[:, bs:be],
                                     rhs=W1s[l + 1][:], start=True, stop=True)
                    tn = sb.tile([P, D], F32, tag="tnsb")
                    nc.vector.tensor_copy(out=tn[:], in_=tn_ps[:])
                    nc.sync.dma_start(out=T_own[l + 1][bs:be, :], in_=tn[:])
            if l < 2:
                nc.gpsimd.collective_compute(
                    "AllGather", mybir.AluOpType.bypass, replica_groups=groups,
                    ins=[T_own[l + 1][:]], outs=[T_full[l + 1][:]])

    nc.compile()
    return nc


def prepare(x, edge_index, Wm0, bm0, Wu0, bu0, Wm1, bm1, Wu1, bu1,
            Wm2, bm2, Wu2, bu2):
    """Returns (nc, in_maps) ready for run_bass_kernel_spmd."""
    x = np.asarray(x, dtype=np.float32)
    edge_index = np.asarray(edge_index)
    gidx, dstl, deg, cnts2, starts2, TOT = _preprocess(edge_index)

    xpad = np.zeros((NPAD, D), dtype=np.float32)
    xpad[:N] = x
    iota = np.broadcast_to(np.arange(P, dtype=np.float32), (P, P)).copy()
    Wm = np.stack([np.asarray(w, np.float32) for w in (Wm0, Wm1, Wm2)])
    Wu = np.stack([np.asarray(w, np.float32) for w in (Wu0, Wu1, Wu2)])
    bm = np.stack([np.asarray(w, np.float32) for w in (bm0, bm1, bm2)])
    bu = np.stack([np.asarray(w, np.float32) for w in (bu0, bu1, bu2)])

    in_maps = []
    for k in range(NCORE):
        deg_k = deg[k * PERCORE:(k + 1) * PERCORE]
        deg2v = np.stack([deg_k, np.ones_like(deg_k)], axis=0)
        in_maps.append({
            "x_own": np.ascontiguousarray(xpad[k * PERCORE:(k + 1) * PERCORE]),
            "gidx": gidx[k],
            "dst_loc": dstl[k],
            "deg2": np.ascontiguousarray(deg2v),
            "iota_in": iota,
            "Wm_in": Wm, "Wu_in": Wu, "bm_in": bm, "bu_in": bu,
        })

    nc = _build(cnts2, starts2, TOT)
    return nc, in_maps


def kernel(**inputs):
    nc, in_maps = prepare(**inputs)
    res = run_bass_kernel_spmd(nc, in_maps, list(range(NCORE)))
    full = np.concatenate([res.results[k]["out"] for k in range(NCORE)], axis=0)
    return np.ascontiguousarray(full[:N])


# revision 9
# speedup vs baseline: 1.8371x; 1.4911x over previous
"""Self-contained Trainium2 Bass kernel for a 3-layer MPNN (N=50000, E=800000, D=64).

Math: each layer is
    x' = relu(concat(segment_sum(x[src]@Wm+bm, dst), x) @ Wu + bu)
with self-loops added. Since the message fn is linear this folds to
    T    = x @ (Wm @ Wu[:D])                      (per-node table)
    y[v] = sum_{e: dst=v} T[src_e]                (scatter-add, no self-loop)
    x'   = relu(y + x @ (Wm@Wu[:D] + Wu[D:]) + deg*(bm@Wu[:D]) + bu)

Sharding: nodes padded to 50176 = 8*6272; core k owns nodes [6272k, 6272(k+1)),
as 49 blocks of 128. Host sorts each core's incident edges by (dst block,
src-half) into 128-edge chunks. Device: batched dma_gather of T rows (one op
per block-half; int16 indices force the src-half table split), one-hot matmul
scatter-add accumulated in PSUM, dense epilogue matmuls, AllGather of the next
layer's T table.
"""
import numpy as np
from contextlib import ExitStack

import concourse.bass as bass
import concourse.bacc as bacc
import concourse.mybir as mybir
import concourse.tile as tile
from concourse.bass_utils import run_bass_kernel_spmd
from concourse.masks import make_identity

N = 50000
E = 800000
D = 64
NCORE = 8
P = 128
PERCORE = 6272          # 49 * 128
NPAD = PERCORE * NCORE  # 50176
NBLK = PERCORE // P     # 49
HALF = 32768            # int16 index limit for dma_gather
F32 = mybir.dt.float32
I16 = mybir.dt.int16


def _preprocess(edge_index):
    """Partition + sort edges by (dst-core, dst-block, src-half).

    Returns:
      gidx  [NCORE, P, 8*TOT]  int16 wrapped gather indices (idx i of a run at
                               [i%16 (replicated mod 16), run_col0 + i//16])
      dstl  [NCORE, P, TOT]    f32 dst-local (edge slot p of chunk col c), -1 pad
      deg   [NPAD]             f32 in-degree + 1
      cnts2 [NBLK, 2]          chunks per (block, half), shared across cores
      starts2 [NBLK, 2]        chunk-column starts
      TOT   total chunk columns
    """
    src = edge_index[0].astype(np.int64)
    dst = edge_index[1].astype(np.int64)
    core = dst // PERCORE
    blk = (dst % PERCORE) // P
    loc = dst % P
    half = (src >= HALF).astype(np.int64)

    deg = np.bincount(dst, minlength=NPAD).astype(np.float32) + 1.0

    order = np.lexsort((loc, half, blk, core))
    src_s, loc_s = src[order], loc[order]
    core_s, blk_s, half_s = core[order], blk[order], half[order]

    cnt = np.zeros((NCORE, NBLK, 2), dtype=np.int64)
    np.add.at(cnt, (core_s, blk_s, half_s), 1)
    cnts2 = (cnt + P - 1) // P
    cnts2 = cnts2.max(axis=0)  # [NBLK, 2]
    flat = cnts2.reshape(-1)
    starts_flat = np.concatenate([[0], np.cumsum(flat)])
    TOT = int(starts_flat[-1])
    starts2 = starts_flat[:-1].reshape(NBLK, 2)

    gidx = np.zeros((NCORE, P, 8 * TOT), dtype=np.int16)
    dstl = np.full((NCORE, P, TOT), -1.0, dtype=np.float32)

    run_start = np.concatenate([[0], np.cumsum(cnt.ravel())])[:-1].reshape(
        NCORE, NBLK, 2)
    for k in range(NCORE):
        for b in range(NBLK):
            for h in range(2):
                n = int(cnt[k, b, h])
                w = int(cnts2[b, h])
                if w == 0:
                    continue
                st = int(starts2[b, h])
                ridx = np.zeros((w * P,), dtype=np.int64)
                rloc = np.full((w * P,), -1.0, dtype=np.float32)
                if n:
                    s0 = int(run_start[k, b, h])
                    ridx[:n] = src_s[s0:s0 + n] - h * HALF
                    rloc[:n] = loc_s[s0:s0 + n]
                # wrapped idx: [16, w*8] -> replicate to 128 partitions
                w16 = ridx.reshape(w * 8, 16).T.astype(np.int16)
                gidx[k, :, 8 * st:8 * (st + w)] = np.tile(w16, (8, 1))
                dstl[k, :, st:st + w] = rloc.reshape(w, P).T
    return gidx, dstl, deg, cnts2.astype(int), starts2.astype(int), TOT


def _build(cnts2, starts2, TOT):
    nc = bacc.Bacc("TRN2", target_bir_lowering=False, debug=False,
                   num_devices=NCORE, num_swdge_queues=4)
    x_own = nc.dram_tensor("x_own", [PERCORE, D], F32, kind="ExternalInput")
    gidx_in = nc.dram_tensor("gidx", [P, 8 * TOT], I16, kind="ExternalInput")
    dst_loc = nc.dram_tensor("dst_loc", [P, TOT], F32, kind="ExternalInput")
    deg2 = nc.dram_tensor("deg2", [2, PERCORE], F32, kind="ExternalInput")
    iota_in = nc.dram_tensor("iota_in", [P, P], F32, kind="ExternalInput")
    Wm_in = nc.dram_tensor("Wm_in", [3, D, D], F32, kind="ExternalInput")
    Wu_in = nc.dram_tensor("Wu_in", [3, 2 * D, D], F32, kind="ExternalInput")
    bm_in = nc.dram_tensor("bm_in", [3, D], F32, kind="ExternalInput")
    bu_in = nc.dram_tensor("bu_in", [3, D], F32, kind="ExternalInput")
    out = nc.dram_tensor("out", [PERCORE, D], F32, kind="ExternalOutput")

    T_own = [nc.dram_tensor(f"T_own{l}", [PERCORE, D], F32) for l in range(3)]
    T_full = [nc.dram_tensor(f"T_full{l}", [NPAD, D], F32, addr_space="Shared")
              for l in range(3)]
    groups = [list(range(NCORE))]

    with tile.TileContext(nc) as tc, ExitStack() as ctx:
        const = ctx.enter_context(tc.tile_pool(name="const", bufs=1))
        sb = ctx.enter_context(tc.tile_pool(name="sb", bufs=4))
        gat = ctx.enter_context(tc.tile_pool(name="gat", bufs=4))
        oneh = ctx.enter_context(tc.tile_pool(name="oneh", bufs=6))
        ps_y = ctx.enter_context(tc.tile_pool(name="ps_y", bufs=2, space="PSUM"))
        ps_m = ctx.enter_context(tc.tile_pool(name="ps_m", bufs=3, space="PSUM"))
        ps_w = ctx.enter_context(tc.tile_pool(name="ps_w", bufs=1, space="PSUM"))

        ident = const.tile([P, P], F32)
        make_identity(nc, ident[:])
        iota = const.tile([P, P], F32)
        nc.sync.dma_start(out=iota[:], in_=iota_in[:])

        gidx_sb = const.tile([P, 8 * TOT], I16, tag="gidx_sb")
        nc.sync.dma_start(out=gidx_sb[:], in_=gidx_in[:])
        dst_all = const.tile([P, TOT], F32, tag="dst_all")
        nc.sync.dma_start(out=dst_all[:], in_=dst_loc[:])
        degt = const.tile([2, PERCORE], F32, tag="degt")
        nc.sync.dma_start(out=degt[:], in_=deg2[:])

        # --- per-layer weight prep: W1 = Wm@Wu_top, W2 = W1 + Wu_bot,
        # bias_rhs = [bm@Wu_top ; bu] ---
        W1s, W2s, biasr = [], [], []
        for l in range(3):
            wm = const.tile([D, D], F32, tag=f"wm{l}")
            nc.sync.dma_start(out=wm[:], in_=Wm_in[l])
            wu_t = const.tile([D, D], F32, tag=f"wut{l}")
            nc.sync.dma_start(out=wu_t[:], in_=Wu_in[l, :D])
            wu_b = const.tile([D, D], F32, tag=f"wub{l}")
            nc.sync.dma_start(out=wu_b[:], in_=Wu_in[l, D:])
            wmT_ps = ps_w.tile([D, D], F32, tag="psw")
            nc.tensor.transpose(out=wmT_ps[:], in_=wm[:], identity=ident[:D, :D])
            wmT = const.tile([D, D], F32, tag=f"wmT{l}")
            nc.vector.tensor_copy(out=wmT[:], in_=wmT_ps[:])
            w1_ps = ps_w.tile([D, D], F32, tag="psw")
            nc.tensor.matmul(out=w1_ps[:], lhsT=wmT[:], rhs=wu_t[:],
                             start=True, stop=True)
            w1 = const.tile([D, D], F32, tag=f"w1{l}")
            nc.vector.tensor_copy(out=w1[:], in_=w1_ps[:])
            w2 = const.tile([D, D], F32, tag=f"w2{l}")
            nc.vector.tensor_add(out=w2[:], in0=w1_ps[:], in1=wu_b[:])
            bmc = const.tile([D, 1], F32, tag=f"bmc{l}")
            nc.sync.dma_start(out=bmc[:], in_=bm_in[l][:, None])
            b1_ps = ps_w.tile([1, D], F32, tag="psw")
            nc.tensor.matmul(out=b1_ps[:], lhsT=bmc[:], rhs=wu_t[:],
                             start=True, stop=True)
            br = const.tile([2, D], F32, tag=f"br{l}")
            nc.vector.tensor_copy(out=br[:1, :], in_=b1_ps[:])
            nc.sync.dma_start(out=br[1:2, :], in_=bu_in[l][None, :])
            W1s.append(w1)
            W2s.append(w2)
            biasr.append(br)

        # persistent transposed-x buffers (ping-pong across layers)
        xT = [const.tile([D, PERCORE], F32, tag=f"xT{i}", name=f"xT{i}")
              for i in range(2)]

        # --- layer 0 table: T0 = x_own @ W1_0 (+ build xT[0]) ---
        for b in range(NBLK):
            bs, be = b * P, (b + 1) * P
            xb = sb.tile([P, D], F32, tag="xb0")
            nc.sync.dma_start(out=xb[:], in_=x_own[bs:be, :])
            xT_ps = ps_m.tile([D, P], F32, tag="psm")
            nc.tensor.transpose(out=xT_ps[:], in_=xb[:], identity=ident[:])
            nc.vector.tensor_copy(out=xT[0][:, bs:be], in_=xT_ps[:])
            t0_ps = ps_m.tile([P, D], F32, tag="psm")
            nc.tensor.matmul(out=t0_ps[:], lhsT=xT[0][:, bs:be], rhs=W1s[0][:],
                             start=True, stop=True)
            t0 = sb.tile([P, D], F32, tag="t0sb")
            nc.vector.tensor_copy(out=t0[:], in_=t0_ps[:])
            nc.sync.dma_start(out=T_own[0][bs:be, :], in_=t0[:])
        nc.gpsimd.collective_compute(
            "AllGather", mybir.AluOpType.bypass, replica_groups=groups,
            ins=[T_own[0][:]], outs=[T_full[0][:]])

        # --- 3 layers ---
        for l in range(3):
            xT_cur = xT[l % 2]
            xT_nxt = xT[(l + 1) % 2]
            for b in range(NBLK):
                bs, be = b * P, (b + 1) * P
                psy = ps_y.tile([P, D], F32, tag="psy")
                first = True
                for h in range(2):
                    c2 = int(cnts2[b, h])
                    if c2 == 0:
                        continue
                    st = int(starts2[b, h])
                    rows = HALF if h == 0 else NPAD - HALF
                    gt = gat.tile([P, c2, D], F32, tag="g")
                    nc.gpsimd.dma_gather(
                        out_ap=gt[:],
                        in_ap=T_full[l][h * HALF:h * HALF + rows, :],
                        idxs_ap=gidx_sb[:, 8 * st:8 * (st + c2)],
                        num_idxs=c2 * P,
                        num_idxs_reg=c2 * P,
                        elem_size=D,
                        single_packet=False,
                        queue_num=(b * 2 + h) % 4,
                    )
                    for c in range(c2):
                        oh = oneh.tile([P, P], F32, tag="oh")
                        nc.vector.tensor_tensor(
                            out=oh[:],
                            in0=dst_all[:, st + c:st + c + 1]
                                .to_broadcast([P, P])[:],
                            in1=iota[:], op=mybir.AluOpType.is_equal)
                        nc.tensor.matmul(out=psy[:], lhsT=oh[:],
                                         rhs=gt[:, c, :],
                                         start=first, stop=False)
                        first = False
                nc.tensor.matmul(out=psy[:], lhsT=xT_cur[:, bs:be],
                                 rhs=W2s[l][:], start=first, stop=False)
                nc.tensor.matmul(out=psy[:], lhsT=degt[:, bs:be],
                                 rhs=biasr[l][:], start=False, stop=True)
                xnew = sb.tile([P, D], F32, tag="xnew")
                nc.scalar.activation(out=xnew[:], in_=psy[:],
                                     func=mybir.ActivationFunctionType.Relu)
                if l == 2:
                    nc.sync.dma_start(out=out[bs:be, :], in_=xnew[:])
                else:
                    xT_ps = ps_m.tile([D, P], F32, tag="psm")
                    nc.tensor.transpose(out=xT_ps[:], in_=xnew[:],
                                        identity=ident[:])
                    nc.vector.tensor_copy(out=xT_nxt[:, bs:be], in_=xT_ps[:])
                    tn_ps = ps_m.tile([P, D], F32, tag="psm")
                    nc.tensor.matmul(out=tn_ps[:], lhsT=xT_nxt[:, bs:be],
                                     rhs=W1s[l + 1][:], start=True, stop=True)
                    tn = sb.tile([P, D], F32, tag="tnsb")
                    nc.vector.tensor_copy(out=tn[:], in_=tn_ps[:])
                    nc.sync.dma_start(out=T_own[l + 1][bs:be, :], in_=tn[:])
            if l < 2:
                nc.gpsimd.collective_compute(
                    "AllGather", mybir.AluOpType.bypass, replica_groups=groups,
                    ins=[T_own[l + 1][:]], outs=[T_full[l + 1][:]])

    nc.compile()
    return nc


def prepare(x, edge_index, Wm0, bm0, Wu0, bu0, Wm1, bm1, Wu1, bu1,
            Wm2, bm2, Wu2, bu2):
    """Returns (nc, in_maps) ready for run_bass_kernel_spmd."""
    x = np.asarray(x, dtype=np.float32)
    edge_index = np.asarray(edge_index)
    gidx, dstl, deg, cnts2, starts2, TOT = _preprocess(edge_index)

    xpad = np.zeros((NPAD, D), dtype=np.float32)
    xpad[:N] = x
    iota = np.broadcast_to(np.arange(P, dtype=np.float32), (P, P)).copy()
    Wm = np.stack([np.asarray(w, np.float32) for w in (Wm0, Wm1, Wm2)])
    Wu = np.stack([np.asarray(w, np.float32) for w in (Wu0, Wu1, Wu2)])
    bm = np.stack([np.asarray(w, np.float32) for w in (bm0, bm1, bm2)])
    bu = np.stack([np.asarray(w, np.float32) for w in (bu0, bu1, bu2)])

    in_maps = []
    for k in range(NCORE):
        deg_k = deg[k * PERCORE:(k + 1) * PERCORE]
        deg2v = np.stack([deg_k, np.ones_like(deg_k)], axis=0)
        in_maps.append({
            "x_own": np.ascontiguousarray(xpad[k * PERCORE:(k + 1) * PERCORE]),
            "gidx": gidx[k],
            "dst_loc": dstl[k],
            "deg2": np.ascontiguousarray(deg2v),
            "iota_in": iota,
            "Wm_in": Wm, "Wu_in": Wu, "bm_in": bm, "bu_in": bu,
        })

    nc = _build(cnts2, starts2, TOT)
    return nc, in_maps


def kernel(**inputs):
    nc, in_maps = prepare(**inputs)
    res = run_bass_kernel_spmd(nc, in_maps, list(range(NCORE)))
    full = np.concatenate([res.results[k]["out"] for k in range(NCORE)], axis=0)
    return np.ascontiguousarray(full[:N])
